# revision 1
# baseline (speedup 1.0000x reference)
"""Gated MLP (SwiGLU) on 8 TRN2 NeuronCores, tensor-parallel over the
intermediate dimension.

Math (per reference): g = x @ Wg.T ; u = x @ Wu.T ; a = silu(g)*u ;
d = a @ Wd.T, with x:[2,2048,4096] f32, Wg/Wu:[14336,4096], Wd:[4096,14336].

Sharding: core c owns intermediate slice I_c = c*1792:(c+1)*1792. Each core
computes gT/uT/aT for its slice against all 4096 tokens, then a partial
dT[c] = WdT[I_c,:].T-contraction. Host sums the 8 partials (the tp_reduce)
and transposes back.

On-chip layout (everything transposed so contractions land on partitions):
  xT  [H=4096, T=4096] bf16            (rhs for gate/up)
  wg/wu [14, 128, 4096] bf16 pre-tiled (lhsT [k128, i128] stationary;
                                        wg[i, p, k*128+m] = Wg.T[k*128+p, i*128+m])
  wd  [32, 128, 1792] bf16 pre-tiled   (lhsT [i128, h128] stationary)
  out [H, T] f32 partial               (dT; host reduces + transposes)

Weight DMAs are contiguous per partition (pre-tiled on host) so each is a
single-block-per-partition SWDGE descriptor set. DMA issue is spread over
engines: xT on Vector, weights on Sync, wd on Scalar, outputs on GpSimd.
"""

import sys

if "/opt/trn_rl_repo" not in sys.path:
    sys.path.insert(0, "/opt/trn_rl_repo")

import numpy as np
import ml_dtypes

H = 4096          # hidden
I_FULL = 14336    # intermediate
T = 4096          # tokens (2*2048)
NCORES = 8
ISH = I_FULL // NCORES   # 1792 per-core intermediate slice
P = 128
QT = 1024         # tokens per outer block
NQ = T // QT      # 4
KT = H // P       # 32 contraction tiles for gate/up
IT = ISH // P     # 14 contraction tiles for down
HT = H // P       # 32 output-row tiles for down
NF = 512          # matmul moving free-dim (one PSUM bank of f32)

_BUILT = {}


def _build():
    if "nc" in _BUILT:
        return _BUILT["nc"]
    from concourse import bacc
    import concourse.mybir as mybir
    import concourse.tile as tile
    from contextlib import ExitStack

    bf = mybir.dt.bfloat16
    f32 = mybir.dt.float32
    nc = bacc.Bacc(
        "TRN2",
        target_bir_lowering=False,
        debug=False,
        enable_asserts=False,
        num_devices=NCORES,
    )

    xT = nc.dram_tensor("xT", [H, T], bf, kind="ExternalInput").ap()
    wg = nc.dram_tensor("wg", [IT, P, KT * P], bf, kind="ExternalInput").ap()
    wu = nc.dram_tensor("wu", [IT, P, KT * P], bf, kind="ExternalInput").ap()
    wd = nc.dram_tensor("wd", [HT, P, IT * P], bf, kind="ExternalInput").ap()
    out = nc.dram_tensor("out", [H, T], f32, kind="ExternalOutput").ap()

    # [p, k, t] view: per-partition rows stay contiguous in t
    x_r = xT.rearrange("(k p) t -> p k t", p=P)     # [128, 32, 4096]

    with tile.TileContext(nc) as tc, ExitStack() as ctx:
        xt_pool = ctx.enter_context(tc.tile_pool(name="xt", bufs=KT + 6))
        wg_pool = ctx.enter_context(tc.tile_pool(name="wg", bufs=2))
        wu_pool = ctx.enter_context(tc.tile_pool(name="wu", bufs=2))
        wd_pool = ctx.enter_context(tc.tile_pool(name="wd", bufs=3))
        at_pool = ctx.enter_context(tc.tile_pool(name="at", bufs=IT + 1))
        tmp_pool = ctx.enter_context(tc.tile_pool(name="tmp", bufs=2))
        dst_pool = ctx.enter_context(tc.tile_pool(name="dst", bufs=3))
        pg_pool = ctx.enter_context(tc.tile_pool(name="pg", bufs=1, space="PSUM"))
        pu_pool = ctx.enter_context(tc.tile_pool(name="pu", bufs=1, space="PSUM"))
        pd_pool = ctx.enter_context(tc.tile_pool(name="pd", bufs=2, space="PSUM"))

        def load_w(pool, src, i):
            t = pool.tile([P, KT, P], bf)
            # src[i] is [128, 4096] contiguous per partition
            nc.scalar.dma_start(out=t[:], in_=src[i].rearrange("p (k m) -> p k m", m=P))
            return t

        for q in range(NQ):
            t0 = q * QT

            # first gate/up weights go out before the xT block so the PE can
            # start as soon as xt[0] lands; at kernel start, stage them in
            # interleaved halves so both wg and wu arrive early
            if q == 0:
                wg_t = wg_pool.tile([P, KT, P], bf)
                wu_t = wu_pool.tile([P, KT, P], bf)
                wgv = wg[0].rearrange("p (k m) -> p k m", m=P)
                wuv = wu[0].rearrange("p (k m) -> p k m", m=P)
                hk = KT // 2
                nc.scalar.dma_start(out=wg_t[:, 0:hk, :], in_=wgv[:, 0:hk, :])
                nc.scalar.dma_start(out=wu_t[:, 0:hk, :], in_=wuv[:, 0:hk, :])
                nc.scalar.dma_start(out=wg_t[:, hk:KT, :], in_=wgv[:, hk:KT, :])
                nc.scalar.dma_start(out=wu_t[:, hk:KT, :], in_=wuv[:, hk:KT, :])
            else:
                wg_t = load_w(wg_pool, wg, 0)
                wu_t = load_w(wu_pool, wu, 0)

            # stage this block's activations: 32 k-tiles of [128, QT]
            xts = []
            for k in range(KT):
                xt_t = xt_pool.tile([P, QT], bf)
                nc.sync.dma_start(out=xt_t[:], in_=x_r[:, k, t0 : t0 + QT])
                xts.append(xt_t)

            # ---- gate/up + silu*mul, producing aT[i] tiles ----
            ats = []
            for i in range(IT):
                if i > 0:
                    wg_t = load_w(wg_pool, wg, i)
                    wu_t = load_w(wu_pool, wu, i)
                pg = pg_pool.tile([P, QT], f32)
                if q == 0 and i == 0:
                    # kernel start: xt tiles arrive at HBM rate (~0.73us per
                    # tile) — interleave g and u per k so PE consumption
                    # (~0.85us/tile) stays behind arrival instead of stalling
                    pu = pu_pool.tile([P, QT], f32)
                    for k in range(KT):
                        for w_t, ps in ((wg_t, pg), (wu_t, pu)):
                            for n in range(QT // NF):
                                nc.tensor.matmul(
                                    ps[:, n * NF : (n + 1) * NF],
                                    w_t[:, k, :],
                                    xts[k][:, n * NF : (n + 1) * NF],
                                    start=(k == 0),
                                    stop=(k == KT - 1),
                                )
                    tmp = tmp_pool.tile([P, QT], bf)
                    nc.scalar.activation(
                        tmp[:], pg[:], mybir.ActivationFunctionType.Silu
                    )
                else:
                    for k in range(KT):
                        for n in range(QT // NF):
                            nc.tensor.matmul(
                                pg[:, n * NF : (n + 1) * NF],
                                wg_t[:, k, :],
                                xts[k][:, n * NF : (n + 1) * NF],
                                start=(k == 0),
                                stop=(k == KT - 1),
                            )
                    # silu(g) on ScalarE while the u matmuls run
                    tmp = tmp_pool.tile([P, QT], bf)
                    nc.scalar.activation(
                        tmp[:], pg[:], mybir.ActivationFunctionType.Silu
                    )
                    pu = pu_pool.tile([P, QT], f32)
                    for k in range(KT):
                        for n in range(QT // NF):
                            nc.tensor.matmul(
                                pu[:, n * NF : (n + 1) * NF],
                                wu_t[:, k, :],
                                xts[k][:, n * NF : (n + 1) * NF],
                                start=(k == 0),
                                stop=(k == KT - 1),
                            )
                at = at_pool.tile([P, QT], bf)
                nc.vector.tensor_tensor(
                    at[:], tmp[:], pu[:], mybir.AluOpType.mult
                )
                ats.append(at)

            # ---- down projection: dT[h, t] partial ----
            for h in range(HT):
                h0 = h * P
                wd_t = wd_pool.tile([P, IT, P], bf)
                nc.gpsimd.dma_start(
                    out=wd_t[:], in_=wd[h].rearrange("p (i m) -> p i m", m=P)
                )
                pd = pd_pool.tile([P, QT], f32)
                for i in range(IT):
                    for n in range(QT // NF):
                        nc.tensor.matmul(
                            pd[:, n * NF : (n + 1) * NF],
                            wd_t[:, i, :],
                            ats[i][:, n * NF : (n + 1) * NF],
                            start=(i == 0),
                            stop=(i == IT - 1),
                        )
                dst = dst_pool.tile([P, QT], f32)
                nc.vector.tensor_copy(dst[:], pd[:])
                # output DMAs on gpsimd so their waits don't stall input loads
                nc.gpsimd.dma_start(
                    out=out[h0 : h0 + P, t0 : t0 + QT], in_=dst[:]
                )

    nc.compile()
    _BUILT["nc"] = nc
    return nc


def _prep_inputs(x, Wg, Wu, Wd):
    bf = ml_dtypes.bfloat16
    xTn = x.reshape(T, H).T.astype(bf, order="C")        # [H, T]
    # single-pass cast + shard + pre-tile:
    #   wg[c][i, p, k*128+m] = Wg.T[k*128+p, c*1792 + i*128+m]
    wg_all = np.ascontiguousarray(
        Wg.reshape(NCORES, IT, P, KT, P).transpose(0, 1, 4, 3, 2), dtype=bf
    ).reshape(NCORES, IT, P, KT * P)
    wu_all = np.ascontiguousarray(
        Wu.reshape(NCORES, IT, P, KT, P).transpose(0, 1, 4, 3, 2), dtype=bf
    ).reshape(NCORES, IT, P, KT * P)
    #   wd[c][h, p, i*128+m] = Wd.T[c*1792 + i*128+p, h*128+m]
    wd_all = np.ascontiguousarray(
        Wd.reshape(HT, P, NCORES, IT, P).transpose(2, 0, 4, 3, 1), dtype=bf
    ).reshape(NCORES, HT, P, IT * P)
    return [
        {"xT": xTn, "wg": wg_all[c], "wu": wu_all[c], "wd": wd_all[c]}
        for c in range(NCORES)
    ]


def _run(in_maps, **kw):
    from concourse.bass_utils import run_bass_kernel_spmd

    nc = _build()
    return run_bass_kernel_spmd(nc, in_maps, core_ids=list(range(NCORES)), **kw)


def _gather(results, batch_shape):
    acc = results[0]["out"].astype(np.float32)
    for r in results[1:]:
        acc += r["out"]
    return np.ascontiguousarray(acc.T).reshape(batch_shape)


def kernel(x, Wg, Wu, Wd):
    x = np.asarray(x)
    in_maps = _prep_inputs(
        np.asarray(x, dtype=np.float32),
        np.asarray(Wg, dtype=np.float32),
        np.asarray(Wu, dtype=np.float32),
        np.asarray(Wd, dtype=np.float32),
    )
    res = _run(in_maps)
    return _gather(res.results, x.shape)

